# revision 1
# baseline (speedup 1.0000x reference)
"""Trainium2 Bass kernel for C2C attention.

Computes, for x:(B,C,T)=(32,64,30000) f32:
    desc = mean(x, axis=2)                       # (B,C)
    q = desc*Wq + bq ; k = desc*Wk + bk          # (B,C,D), D=64
    attn = softmax(q @ k^T / sqrt(D))            # (B,C,C)
    out = x + alpha * attn @ x
      == (I + alpha*attn) @ x                    # folded residual

Sharding: pure data parallel over batch, 4 batches per core on 8 cores.
On each core, batches are processed as 2 "pairs"; a pair stacks two
batches on the 128 SBUF partitions and uses a block-diagonal 128x128
stationary matrix (I + alpha*attn_b0 (+) I + alpha*attn_b1)^T so one
matmul pass computes both batches.  The big matmul runs in FP32R
(single-pass full-rate fp32) and its result is written back in place
over the consumed x segment, then DMA'd out.
"""

import os

import numpy as np

import concourse.bass as bass
import concourse.tile as tile
from concourse import bacc, mybir
from concourse.bass_utils import run_bass_kernel_spmd


B, C, T, D = 32, 64, 30000, 64
N_CORES = 8
BPC = B // N_CORES          # batches per core = 4
PAIRS = BPC // 2            # 2
ROWS = BPC * C              # 256 rows of (row, T) per core
SEG = 3000                  # columns per DMA segment
NSEG = T // SEG             # 10
CHUNK = 500                 # matmul moving free dim (<=512, fits PSUM bank)
GROUP = 2                   # chunks per PSUM tile (2 banks) -> 1000-col copies
NGRP = SEG // (CHUNK * GROUP)   # 3 groups per segment
XBUFS = 15                  # xseg ring slots (5 slots of cross-pair slack)
SPARE = XBUFS - NSEG        # pair1 segs loadable before pair0 slots free

F32 = mybir.dt.float32
F32R = mybir.dt.float32r    # single-pass full-rate fp32 matmul (moving dim>=256)
AX = mybir.AxisListType
AF = mybir.ActivationFunctionType

# packed constants layout, one (128, 513) f32 block:
#   [:, 0:128]    identity(128)
#   [:, 128:129]  alpha broadcast
#   [0:2, 129:193]   [Wq/(8T); bq/8]
#   [0:2, 193:257]   [Wk/T;   bk  ]
#   [0:2, 257:385]   qk-matmul rhs init: row0 = 0 (sums placeholder), row1 = 1
#   [:, 385:513]  zeros -> attn scratch (off-diagonal blocks must stay 0)
CONST_COLS = 513


def build_bass() -> bass.Bass:
    nc = bacc.Bacc()

    # x is stored/streamed as float32r (same bits as f32): the main matmul
    # runs in single-pass FP32R mode, which requires f32r-typed operands.
    x = nc.dram_tensor("x", [ROWS, T], F32R, kind="ExternalInput")
    out = nc.dram_tensor("out", [ROWS, T], F32, kind="ExternalOutput")
    consts_d = nc.dram_tensor("consts", [128, CONST_COLS], F32,
                              kind="ExternalInput")

    with tile.TileContext(nc) as tc, \
            tc.tile_pool(name="consts", bufs=1) as consts, \
            tc.tile_pool(name="pairbuf", bufs=2) as pairbuf, \
            tc.tile_pool(name="xsegs", bufs=XBUFS) as xsegs, \
            tc.tile_pool(name="psmm", bufs=3, space="PSUM") as psmm, \
            tc.tile_pool(name="pssm", bufs=2, space="PSUM") as pssm:

        cblk = consts.tile([128, CONST_COLS], F32)
        nc.sync.dma_start(out=cblk, in_=consts_d[:, :])
        ident = cblk[:, 0:128]
        alpha_bc = cblk[:, 128:129]
        wq2 = cblk[0:2, 129:193]
        wk2 = cblk[0:2, 193:257]
        rhs_qk = cblk[0:2, 257:385]
        attn = cblk[:, 385:513]
        scratch = consts.tile([128, 1], F32)
        # pre-load the ACT exp table off the critical path
        nc.scalar.activation(out=scratch, in_=alpha_bc, func=AF.Exp)

        xs = [[None] * NSEG for _ in range(PAIRS)]
        partials = [None] * PAIRS
        lhsT = [None] * PAIRS

        last_in_dma = [None] * PAIRS

        def emit_load_seg(p, s):
            xt = xsegs.tile([128, SEG], F32R, tag="xseg")
            xs[p][s] = xt
            last_in_dma[p] = nc.sync.dma_start(
                out=xt,
                in_=x[p * 128:(p + 1) * 128, s * SEG:(s + 1) * SEG],
            )
            nc.vector.reduce_sum(out=partials[p][:, s:s + 1],
                                 in_=xt.bitcast(F32), axis=AX.X)

        def emit_load_reduce(p, segs):
            if partials[p] is None:
                part = pairbuf.tile([128, NSEG], F32, tag="partial")
                partials[p] = part
            for s in segs:
                emit_load_seg(p, s)

        def emit_smalls(p):
            # sums over T for both batches of the pair: (128,1)
            sums = pairbuf.tile([128, 1], F32, tag="sums")
            nc.vector.reduce_sum(out=sums, in_=partials[p], axis=AX.X)
            # transpose to a row: (1,128)
            srow_ps = pssm.tile([1, 128], F32, tag="ps_small")
            nc.tensor.transpose(out=srow_ps, in_=sums, identity=ident)
            nc.scalar.copy(out=rhs_qk[0:1, :], in_=srow_ps)
            # qT/kT = [w; b]^T @ [sums_row; ones] : (D, 2C) covering both batches
            qT_ps = pssm.tile([D, 2 * C], F32, tag="ps_small")
            nc.tensor.matmul(out=qT_ps, lhsT=wq2, rhs=rhs_qk, start=True, stop=True)
            qT = pairbuf.tile([D, 2 * C], F32, tag="qT")
            nc.scalar.copy(out=qT, in_=qT_ps)
            kT_ps = pssm.tile([D, 2 * C], F32, tag="ps_small")
            nc.tensor.matmul(out=kT_ps, lhsT=wk2, rhs=rhs_qk, start=True, stop=True)
            kT = pairbuf.tile([D, 2 * C], F32, tag="kT")
            nc.scalar.copy(out=kT, in_=kT_ps)
            # logits for both batches on the diagonal blocks of (128,128)
            lg_ps = pssm.tile([128, 128], F32, tag="ps_small")
            nc.tensor.matmul(out=lg_ps, lhsT=qT, rhs=kT, start=True, stop=True)
            # exp of each diagonal block; accum_out gives the softmax denominator
            sumexp = pairbuf.tile([128, 1], F32, tag="sumexp")
            for h in range(2):
                r = slice(h * 64, h * 64 + 64)
                nc.scalar.activation(
                    out=attn[r, r], in_=lg_ps[r, r], func=AF.Exp,
                    accum_out=sumexp[r, :],
                )
            recip = pairbuf.tile([128, 1], F32, tag="recip")
            nc.vector.reciprocal(out=recip, in_=sumexp)
            nc.vector.tensor_scalar(out=attn, in0=attn, scalar1=recip,
                                    scalar2=alpha_bc,
                                    op0=mybir.AluOpType.mult,
                                    op1=mybir.AluOpType.mult)
            # lhsT = (I + alpha*attn)^T = I + (alpha*attn)^T
            at_ps = pssm.tile([128, 128], F32, tag="ps_small")
            nc.tensor.transpose(out=at_ps, in_=attn, identity=ident)
            lt = pairbuf.tile([128, 128], F32, tag="lhsT")
            nc.vector.tensor_add(out=lt, in0=at_ps, in1=ident)
            # round the stationary operand to f32r for the FP32R matmul
            ltr = pairbuf.tile([128, 128], F32R, tag="lhsTr")
            nc.scalar.copy(out=ltr, in_=lt)
            lhsT[p] = ltr

        def emit_compute(p):
            for s in range(NSEG):
                xt = xs[p][s]
                for g in range(NGRP):
                    mm = psmm.tile([128, GROUP, 512], F32, tag="mm")
                    base = g * GROUP * CHUNK
                    for j in range(GROUP):
                        nc.tensor.matmul(
                            out=mm[:, j, 0:CHUNK],
                            lhsT=lhsT[p],
                            rhs=xt[:, base + j * CHUNK: base + (j + 1) * CHUNK],
                            start=True, stop=True,
                        )
                    dst = xt[:, base: base + GROUP * CHUNK].rearrange(
                        "p (a c) -> p a c", a=GROUP)
                    nc.scalar.copy(out=dst, in_=mm[:, :, 0:CHUNK])

        def emit_out(p, segs, hold_for=None):
            for s in segs:
                odma = nc.sync.dma_start(
                    out=out[p * 128:(p + 1) * 128, s * SEG:(s + 1) * SEG],
                    in_=xs[p][s].bitcast(F32),
                )
                if hold_for is not None:
                    # reserve this output traffic for the window where the
                    # next pair's attention chain runs (queued transfers share
                    # the SDMA engines, so only a hard dep can hold it back)
                    tile.add_dep_helper(hold_for.ins, odma.ins, sync=True,
                                        reason="reserve out traffic")

        # Phase schedule (all DMAs on the SP HWDGE ring; emission order is
        # trigger order):  in0 | in1[0:5] | out0[0:5] | in1[5:10] |
        # out0[5:10] held until in1 done | out1.  The held 5 segments keep
        # the DMA busy while pair1's attention chain + first copies run.
        emit_load_reduce(0, range(NSEG))
        emit_smalls(0)
        emit_load_reduce(1, range(SPARE))
        emit_compute(0)
        emit_out(0, range(SPARE))
        emit_load_reduce(1, range(SPARE, NSEG))
        emit_out(0, range(SPARE, NSEG), hold_for=last_in_dma[1])
        emit_smalls(1)
        emit_compute(1)
        emit_out(1, range(NSEG))

    # Bacc legalization: splits multi-wait sync into EventSemaphore
    # instructions (HW allows one wait per instruction) etc.
    nc.compile()
    return nc


def _host_inputs(x, Wq, bq, Wk, bk, Wv, bv, alpha):
    """Build per-core input maps. Scale folding:
    logits[c,e] = (q[c]/8) . k[e],  q/8 = (Wq/(8T))*sums + bq/8, k = (Wk/T)*sums + bk
    """
    x = np.ascontiguousarray(np.asarray(x, dtype=np.float32))
    cb = np.zeros((128, CONST_COLS), dtype=np.float32)
    cb[:, 0:128] = np.eye(128, dtype=np.float32)
    cb[:, 128] = np.float32(alpha)
    cb[0, 129:193] = np.asarray(Wq)[:, 0] / (8.0 * T)
    cb[1, 129:193] = np.asarray(bq) / 8.0
    cb[0, 193:257] = np.asarray(Wk)[:, 0] / T
    cb[1, 193:257] = np.asarray(bk)
    cb[1, 257:385] = 1.0
    in_maps = []
    for c in range(N_CORES):
        shard = x[c * BPC:(c + 1) * BPC].reshape(ROWS, T)
        in_maps.append({
            "x": np.ascontiguousarray(shard),
            "consts": cb,
        })
    return in_maps


def run(inputs: dict, trace: bool = False, tmpdir: str | None = None):
    nc = build_bass()
    in_maps = _host_inputs(**inputs)
    res = run_bass_kernel_spmd(
        nc, in_maps, core_ids=list(range(N_CORES)), trace=trace, tmpdir=tmpdir,
    )
    outs = [m["out"].reshape(BPC, C, T) for m in res.results]
    full = np.concatenate(outs, axis=0)
    return full, res


def kernel(**inputs) -> np.ndarray:
    full, _ = run(inputs, trace=bool(os.environ.get("C2C_TRACE")))
    return full


if __name__ == "__main__":
    # quick single-core numerical check in CoreSim
    from concourse import bass_interp

    rng = np.random.default_rng(0)
    x = rng.standard_normal((BPC, C, T), dtype=np.float32)
    Wq = rng.standard_normal((D, 1)).astype(np.float32)
    bq = rng.standard_normal((D,)).astype(np.float32)
    Wk = rng.standard_normal((D, 1)).astype(np.float32)
    bk = rng.standard_normal((D,)).astype(np.float32)
    alpha = np.float32(0.5)

    nc = build_bass()
    sim = bass_interp.CoreSim(nc)
    im = _host_inputs(x=np.tile(x, (N_CORES, 1, 1)), Wq=Wq, bq=bq, Wk=Wk, bk=bk,
                      Wv=None, bv=None, alpha=alpha)[0]
    for k, v in im.items():
        sim.tensor(k)[:] = v
    sim.simulate()
    got = np.asarray(sim.tensor("out")).reshape(BPC, C, T)

    desc = x.mean(axis=2, keepdims=True)
    q = desc * Wq[:, 0] + bq
    k = desc * Wk[:, 0] + bk
    logits = np.einsum('bcd,bed->bce', q, k) / np.sqrt(D)
    m = logits.max(axis=-1, keepdims=True)
    e = np.exp(logits - m)
    attn = e / e.sum(axis=-1, keepdims=True)
    mixed = np.einsum('bce,bet->bct', attn, x)
    want = x + alpha * mixed
    err = np.abs(got - want)
    rel = np.linalg.norm(got - want) / np.linalg.norm(want)
    print("max abs err:", err.max(), "rel:", rel)



# revision 13
# speedup vs baseline: 1.0622x; 1.0622x over previous
"""Trainium2 Bass kernel for C2C attention.

Computes, for x:(B,C,T)=(32,64,30000) f32:
    desc = mean(x, axis=2)                       # (B,C)
    q = desc*Wq + bq ; k = desc*Wk + bk          # (B,C,D), D=64
    attn = softmax(q @ k^T / sqrt(D))            # (B,C,C)
    out = x + alpha * attn @ x
      == (I + alpha*attn) @ x                    # folded residual

Sharding: pure data parallel over batch, 4 batches per core on 8 cores.
Each core processes 2 "pairs"; a pair stacks two batches on the 128 SBUF
partitions and uses a block-diagonal (I + alpha*attn)^T stationary matrix
so one FP32R matmul pass computes both batches.

Schedule (v2): loads ride the SP HWDGE ring, stores ride the ACT HWDGE
ring; the 16 SDMA engines round-robin between rings at packet
granularity, so queued store traffic fills any load-side gap and vice
versa.  Pair0 lives in SBUF region A ([128,30000]); pair1 cols 0-18000
in region B, cols 18000-30000 reuse A[:, 0:12000] after pair0's first
two stores drain.  The logits are rank-2 in the channel sums
(logits = a1*s s^T + a2*s 1^T + a3*1 s^T + a4), so the attention chain
is one tiny matmul instead of q/k matmuls, and +I is folded into the
PSUM transpose by an accumulating identity matmul.
"""

import os

import numpy as np

import concourse.bass as bass
import concourse.tile as tile
from concourse import bacc, mybir
from concourse.bass_utils import run_bass_kernel_spmd


B, C, T, D = 32, 64, 30000, 64
N_CORES = 8
BPC = B // N_CORES          # batches per core = 4
PAIRS = BPC // 2            # 2
ROWS = BPC * C              # 256 rows of (row, T) per core
TB = 18000                  # region B cols; pair1 cols [TB:T] live in A[:, 0:T-TB]
CHUNK = 500                 # matmul moving free dim (<=512, fits PSUM bank)
GROUP = 2                   # chunks per PSUM tile (2 banks) -> 1000-col copies
GCOLS = CHUNK * GROUP       # 1000

# load segmentation: last segment small so the latency-critical final
# reduce on Vector is short.
IN0 = [(0, 7500), (7500, 15000), (15000, 22500), (22500, 28800), (28800, 30000)]
IN1B = [(0, 7500), (7500, 15000), (15000, 18000)]
IN1A = [(0, 6000), (6000, 9600), (9600, 12000)]     # pair1 cols +18000
OUT0 = [(0, 6000), (6000, 12000), (12000, 18000), (18000, 24000), (24000, 30000)]
OUT1B = [(0, 3000), (3000, 10000), (10000, 18000)]  # first store small: early start
OUT1A = [(0, 6000), (6000, 12000)]                  # pair1 cols +18000

F32 = mybir.dt.float32
F32R = mybir.dt.float32r    # single-pass full-rate fp32 matmul (moving dim>=256)
AX = mybir.AxisListType
AF = mybir.ActivationFunctionType
MUL = mybir.AluOpType.mult
ADD = mybir.AluOpType.add

# packed constants layout, one (128, 528) f32 block:
#   [:, 0:128]    identity(128)
#   [:, 128:129]  alpha broadcast
#   [0,129] a1   [1,129] a2   [0,130] a3   [1,130] a4   (rank-2 logit coeffs)
#   [0:2, 144:272]  rhs_qk: row1 = 1 (host), row0 = sums row (runtime)
#   [0:2, 272:400]  lhsT2 scratch (runtime)
#   [:, 400:528]  zeros -> attn scratch (off-diagonal blocks must stay 0)
CONST_COLS = 528


def build_bass() -> bass.Bass:
    nc = bacc.Bacc()

    # x is stored/streamed as float32r (same bits as f32): the main matmul
    # runs in single-pass FP32R mode, which requires f32r-typed operands.
    x = nc.dram_tensor("x", [ROWS, T], F32R, kind="ExternalInput")
    out = nc.dram_tensor("out", [ROWS, T], F32, kind="ExternalOutput")
    consts_d = nc.dram_tensor("consts", [128, CONST_COLS], F32,
                              kind="ExternalInput")

    with tile.TileContext(nc) as tc, \
            tc.tile_pool(name="consts", bufs=1) as consts, \
            tc.tile_pool(name="xbuf", bufs=1) as xbuf, \
            tc.tile_pool(name="pairbuf", bufs=2) as pairbuf, \
            tc.tile_pool(name="psmm", bufs=3, space="PSUM") as psmm, \
            tc.tile_pool(name="pssm", bufs=2, space="PSUM") as pssm:

        A = xbuf.tile([128, T], F32R, tag="A")
        Bt = xbuf.tile([128, TB], F32R, tag="B")
        Af = A.bitcast(F32)
        Bf = Bt.bitcast(F32)

        cblk = consts.tile([128, CONST_COLS], F32)
        ident = cblk[:, 0:128]
        alpha_bc = cblk[:, 128:129]
        a12c = cblk[0:2, 129:130]   # (a1; a2) per-partition
        a34c = cblk[0:2, 130:131]   # (a3; a4) per-partition
        rhs_qk = cblk[0:2, 144:272]
        lhsT2 = cblk[0:2, 272:400]
        attn = cblk[:, 400:528]

        part0 = pairbuf.tile([128, len(IN0)], F32, tag="part0")
        part1 = pairbuf.tile([128, len(IN1B) + len(IN1A)], F32, tag="part1")
        partials = [part0, part1]
        lhsT = [None] * PAIRS

        def emit_loads(segs, dst, dstf, drow, dcol_off, part, pcol0, eng=None):
            for i, (c0, c1) in enumerate(segs):
                (eng or nc.sync).dma_start(
                    out=dst[:, c0:c1],
                    in_=x[drow:drow + 128, dcol_off + c0:dcol_off + c1])
                nc.vector.reduce_sum(out=part[:, pcol0 + i:pcol0 + i + 1],
                                     in_=dstf[:, c0:c1], axis=AX.X)

        def emit_smalls(p):
            # channel sums duplicated into two columns so the transpose
            # yields the sums row on both partitions 0 and 1
            sums2 = pairbuf.tile([128, 2], F32, tag="sums2")
            nc.vector.reduce_sum(out=sums2[:, 0:1], in_=partials[p], axis=AX.X)
            nc.vector.reduce_sum(out=sums2[:, 1:2], in_=partials[p], axis=AX.X)
            srow2 = pssm.tile([2, 128], F32, tag="ps_small")
            nc.tensor.transpose(out=srow2, in_=sums2, identity=ident)
            # logits = (a1*s + a3 ; a2*s + a4)^T @ (s ; 1)  -- rank-2 in s
            nc.scalar.copy(out=rhs_qk[0:1, :], in_=srow2[0:1, :])
            nc.vector.tensor_scalar(out=lhsT2, in0=srow2,
                                    scalar1=a12c, scalar2=a34c,
                                    op0=MUL, op1=ADD)
            lg_ps = pssm.tile([128, 128], F32, tag="ps_small")
            nc.tensor.matmul(out=lg_ps, lhsT=lhsT2, rhs=rhs_qk,
                             start=True, stop=True)
            # exp of each diagonal block; accum_out gives the softmax denom
            sumexp = pairbuf.tile([128, 1], F32, tag="sumexp")
            for h in range(2):
                r = slice(h * 64, h * 64 + 64)
                nc.scalar.activation(
                    out=attn[r, r], in_=lg_ps[r, r], func=AF.Exp,
                    accum_out=sumexp[r, :],
                )
            recip = pairbuf.tile([128, 1], F32, tag="recip")
            nc.vector.reciprocal(out=recip, in_=sumexp)
            nc.vector.tensor_scalar(out=attn, in0=attn, scalar1=recip,
                                    scalar2=alpha_bc, op0=MUL, op1=MUL)
            # lhsT = (I + alpha*attn)^T: transpose into PSUM, then
            # accumulate identity (ident^T @ ident = I) into the same bank
            at_ps = pssm.tile([128, 128], F32, tag="ps_small")
            nc.tensor.transpose(out=at_ps, in_=attn, identity=ident)
            lt = pairbuf.tile([128, 128], F32, tag="lhsTf")
            nc.vector.tensor_add(out=lt, in0=at_ps, in1=ident)
            ltr = pairbuf.tile([128, 128], F32R, tag="lhsTr")
            nc.scalar.copy(out=ltr, in_=lt)
            lhsT[p] = ltr

        def emit_compute(p, src, srcf, ncols, stores, drow, dcol_off,
                         after_group=None):
            # stores: list of (end_group, c0, c1); store k emitted right
            # after its covering copy-group lands
            ngrp = ncols // GCOLS
            by_group = {g: (c0, c1) for (g, c0, c1) in stores}
            for g in range(ngrp):
                if after_group and g in after_group:
                    after_group[g]()
                mm = psmm.tile([128, GROUP, 512], F32, tag="mm")
                base = g * GCOLS
                for j in range(GROUP):
                    nc.tensor.matmul(
                        out=mm[:, j, 0:CHUNK],
                        lhsT=lhsT[p],
                        rhs=src[:, base + j * CHUNK: base + (j + 1) * CHUNK],
                        start=True, stop=True,
                    )
                dst = src[:, base: base + GCOLS].rearrange(
                    "p (a c) -> p a c", a=GROUP)
                nc.scalar.copy(out=dst, in_=mm[:, :, 0:CHUNK])
                if g in by_group:
                    c0, c1 = by_group[g]
                    nc.scalar.dma_start(
                        out=out[drow:drow + 128, dcol_off + c0:dcol_off + c1],
                        in_=srcf[:, c0:c1])

        # ---- emission: defines per-engine queue order ----
        # pair0 loads (SP ring), consts after the first so x data leads
        c0, c1 = IN0[0]
        nc.sync.dma_start(out=A[:, c0:c1], in_=x[0:128, c0:c1])
        nc.sync.dma_start(out=cblk, in_=consts_d[:, :])
        nc.vector.reduce_sum(out=partials[0][:, 0:1], in_=Af[:, c0:c1],
                             axis=AX.X)
        scratch = consts.tile([128, 1], F32)
        # pre-load the ACT exp table off the critical path
        nc.scalar.activation(out=scratch, in_=alpha_bc, func=AF.Exp)
        for i, (c0, c1) in enumerate(IN0[1:], start=1):
            nc.sync.dma_start(out=A[:, c0:c1], in_=x[0:128, c0:c1])
            nc.vector.reduce_sum(out=partials[0][:, i:i + 1],
                                 in_=Af[:, c0:c1], axis=AX.X)
        emit_smalls(0)
        # pair1 B-region loads: region free immediately
        emit_loads(IN1B, Bt, Bf, 128, 0, partials[1], 0)

        # pair1 A-region loads ride the ACT ring, emitted right after
        # out0[1]'s trigger: same-ring FIFO guarantees their writes to
        # A[:, 0:12000] happen after pair0's stores read those columns
        # (cross-ring DMA WAR is NOT tracked by Tile).
        def emit_in1a():
            emit_loads(IN1A, A, Af, 128, TB, partials[1], len(IN1B),
                       eng=nc.scalar)

        # pair0 compute + stores (ACT ring)
        emit_compute(0, A, Af, T,
                     [(o1 // GCOLS - 1, o0, o1) for (o0, o1) in OUT0],
                     0, 0, after_group={12: emit_in1a})
        emit_smalls(1)
        # pair1 compute: B region, then the A-region tail
        emit_compute(1, Bt, Bf, TB,
                     [(o1 // GCOLS - 1, o0, o1) for (o0, o1) in OUT1B],
                     128, 0)
        emit_compute(1, A, Af, T - TB,
                     [(o1 // GCOLS - 1, o0, o1) for (o0, o1) in OUT1A],
                     128, TB)

    # Bacc legalization: splits multi-wait sync into EventSemaphore
    # instructions (HW allows one wait per instruction) etc.
    nc.compile()
    return nc


def _host_inputs(x, Wq, bq, Wk, bk, Wv, bv, alpha):
    """Build per-core input maps.  Rank-2 logit coefficients, with the
    1/sqrt(D)=1/8 scale and the 1/T mean folded in:
    logits[c,e] = a1*S_c*S_e + a2*S_c + a3*S_e + a4, S = row sums of x.
    """
    x = np.ascontiguousarray(np.asarray(x, dtype=np.float32))
    wq = np.asarray(Wq)[:, 0].astype(np.float64)
    wk = np.asarray(Wk)[:, 0].astype(np.float64)
    bqv = np.asarray(bq).astype(np.float64)
    bkv = np.asarray(bk).astype(np.float64)
    cb = np.zeros((128, CONST_COLS), dtype=np.float32)
    cb[:, 0:128] = np.eye(128, dtype=np.float32)
    cb[:, 128] = np.float32(alpha)
    cb[0, 129] = wq @ wk / (8.0 * T * T)
    cb[1, 129] = wq @ bkv / (8.0 * T)
    cb[0, 130] = bqv @ wk / (8.0 * T)
    cb[1, 130] = bqv @ bkv / 8.0
    cb[1, 144:272] = 1.0
    in_maps = []
    for c in range(N_CORES):
        shard = x[c * BPC:(c + 1) * BPC].reshape(ROWS, T)
        in_maps.append({
            "x": np.ascontiguousarray(shard),
            "consts": cb,
        })
    return in_maps


def run(inputs: dict, trace: bool = False, tmpdir: str | None = None):
    nc = build_bass()
    in_maps = _host_inputs(**inputs)
    res = run_bass_kernel_spmd(
        nc, in_maps, core_ids=list(range(N_CORES)), trace=trace, tmpdir=tmpdir,
    )
    outs = [m["out"].reshape(BPC, C, T) for m in res.results]
    full = np.concatenate(outs, axis=0)
    return full, res


def kernel(**inputs) -> np.ndarray:
    full, _ = run(inputs, trace=bool(os.environ.get("C2C_TRACE")))
    return full


if __name__ == "__main__":
    # quick single-core numerical check in CoreSim
    from concourse import bass_interp

    rng = np.random.default_rng(0)
    x = rng.standard_normal((BPC, C, T), dtype=np.float32)
    Wq = rng.standard_normal((D, 1)).astype(np.float32)
    bq = rng.standard_normal((D,)).astype(np.float32)
    Wk = rng.standard_normal((D, 1)).astype(np.float32)
    bk = rng.standard_normal((D,)).astype(np.float32)
    alpha = np.float32(0.5)

    nc = build_bass()
    sim = bass_interp.CoreSim(nc)
    im = _host_inputs(x=np.tile(x, (N_CORES, 1, 1)), Wq=Wq, bq=bq, Wk=Wk, bk=bk,
                      Wv=None, bv=None, alpha=alpha)[0]
    for k, v in im.items():
        sim.tensor(k)[:] = v
    sim.simulate()
    got = np.asarray(sim.tensor("out")).reshape(BPC, C, T)

    desc = x.mean(axis=2, keepdims=True)
    q = desc * Wq[:, 0] + bq
    k = desc * Wk[:, 0] + bk
    logits = np.einsum('bcd,bed->bce', q, k) / np.sqrt(D)
    m = logits.max(axis=-1, keepdims=True)
    e = np.exp(logits - m)
    attn = e / e.sum(axis=-1, keepdims=True)
    mixed = np.einsum('bce,bet->bct', attn, x)
    want = x + alpha * mixed
    err = np.abs(got - want)
    rel = np.linalg.norm(got - want) / np.linalg.norm(want)
    print("max abs err:", err.max(), "rel:", rel)


# revision 17
# speedup vs baseline: 1.1405x; 1.0737x over previous
"""Trainium2 Bass kernel for C2C attention.

Computes, for x:(B,C,T)=(32,64,30000) f32:
    desc = mean(x, axis=2)                       # (B,C)
    q = desc*Wq + bq ; k = desc*Wk + bk          # (B,C,D), D=64
    attn = softmax(q @ k^T / sqrt(D))            # (B,C,C)
    out = x + alpha * attn @ x
      == (I + alpha*attn) @ x                    # folded residual

Sharding: pure data parallel over batch, 4 batches per core on 8 cores.
Each core processes 2 "pairs"; a pair stacks two batches on the 128 SBUF
partitions and uses a block-diagonal (I + alpha*attn)^T stationary matrix
so one FP32R matmul pass computes both batches.

Schedule (v2): loads ride the SP HWDGE ring, stores ride the ACT HWDGE
ring; the 16 SDMA engines round-robin between rings at packet
granularity, so queued store traffic fills any load-side gap and vice
versa.  Pair0 lives in SBUF region A ([128,30000]); pair1 cols 0-18000
in region B, cols 18000-30000 reuse A[:, 0:12000] after pair0's first
two stores drain.  The logits are rank-2 in the channel sums
(logits = a1*s s^T + a2*s 1^T + a3*1 s^T + a4), so the attention chain
is one tiny matmul instead of q/k matmuls, and +I is folded into the
PSUM transpose by an accumulating identity matmul.
"""

import os

import numpy as np

import concourse.bass as bass
import concourse.tile as tile
from concourse import bacc, mybir
from concourse.bass_utils import run_bass_kernel_spmd


B, C, T, D = 32, 64, 30000, 64
N_CORES = 8
BPC = B // N_CORES          # batches per core = 4
PAIRS = BPC // 2            # 2
ROWS = BPC * C              # 256 rows of (row, T) per core
TB = 21000                  # region B cols; pair1 cols [TB:T] live in A[:, 0:T-TB]
CHUNK = 500                 # matmul moving free dim (<=512, fits PSUM bank)
GROUP = 2                   # chunks per PSUM tile (2 banks) -> 1000-col copies
GCOLS = CHUNK * GROUP       # 1000

# load segmentation: last segment small so the latency-critical final
# reduce on Vector is short.
IN0 = [(0, 7500), (7500, 15000), (15000, 22500), (22500, 28800), (28800, 30000)]
IN1B = [(0, 7500), (7500, 15000), (15000, 21000)]
IN1A = [(0, 6000), (6000, 8000), (8000, 9000)]      # pair1 cols +21000
OUT0 = [(0, 6000), (6000, 12000), (12000, 18000), (18000, 24000),
        (24000, 27000), (27000, 30000)]
OUT1B = [(0, 2000), (2000, 11000), (11000, 21000)]  # first store small: early start
OUT1A = [(0, 6000), (6000, 9000)]                   # pair1 cols +21000

F32 = mybir.dt.float32
F32R = mybir.dt.float32r    # single-pass full-rate fp32 matmul (moving dim>=256)
AX = mybir.AxisListType
AF = mybir.ActivationFunctionType
MUL = mybir.AluOpType.mult
ADD = mybir.AluOpType.add

# packed constants layout, one (128, 528) f32 block:
#   [:, 0:128]    identity(128)
#   [:, 128:129]  alpha broadcast
#   [0,129] a1   [1,129] a2   [0,130] a3   [1,130] a4   (rank-2 logit coeffs)
#   [0:2, 144:272]  rhs_qk: row1 = 1 (host), row0 = sums row (runtime)
#   [0:2, 272:400]  lhsT2 scratch (runtime)
#   [:, 400:528]  zeros -> attn scratch (off-diagonal blocks must stay 0)
CONST_COLS = 528


def build_bass() -> bass.Bass:
    nc = bacc.Bacc()

    # x is stored/streamed as float32r (same bits as f32): the main matmul
    # runs in single-pass FP32R mode, which requires f32r-typed operands.
    x = nc.dram_tensor("x", [ROWS, T], F32R, kind="ExternalInput")
    out = nc.dram_tensor("out", [ROWS, T], F32, kind="ExternalOutput")
    consts_d = nc.dram_tensor("consts", [128, CONST_COLS], F32,
                              kind="ExternalInput")

    with tile.TileContext(nc) as tc, \
            tc.tile_pool(name="consts", bufs=1) as consts, \
            tc.tile_pool(name="xbuf", bufs=1) as xbuf, \
            tc.tile_pool(name="pairbuf", bufs=2) as pairbuf, \
            tc.tile_pool(name="psmm", bufs=3, space="PSUM") as psmm, \
            tc.tile_pool(name="pssm", bufs=2, space="PSUM") as pssm:

        A = xbuf.tile([128, T], F32R, tag="A")
        Bt = xbuf.tile([128, TB], F32R, tag="B")
        Af = A.bitcast(F32)
        Bf = Bt.bitcast(F32)

        cblk = consts.tile([128, CONST_COLS], F32)
        ident = cblk[:, 0:128]
        alpha_bc = cblk[:, 128:129]
        a12c = cblk[0:2, 129:130]   # (a1; a2) per-partition
        a34c = cblk[0:2, 130:131]   # (a3; a4) per-partition
        rhs_qk = cblk[0:2, 144:272]
        lhsT2 = cblk[0:2, 272:400]
        attn = cblk[:, 400:528]

        part0 = pairbuf.tile([128, len(IN0)], F32, tag="part0")
        part1 = pairbuf.tile([128, len(IN1B) + len(IN1A)], F32, tag="part1")
        partials = [part0, part1]
        lhsT = [None] * PAIRS

        def emit_loads(segs, dst, dstf, drow, dcol_off, part, pcol0, eng=None):
            for i, (c0, c1) in enumerate(segs):
                (eng or nc.sync).dma_start(
                    out=dst[:, c0:c1],
                    in_=x[drow:drow + 128, dcol_off + c0:dcol_off + c1])
                nc.vector.reduce_sum(out=part[:, pcol0 + i:pcol0 + i + 1],
                                     in_=dstf[:, c0:c1], axis=AX.X)

        def emit_smalls(p):
            # channel sums duplicated into two columns so the transpose
            # yields the sums row on both partitions 0 and 1
            sums2 = pairbuf.tile([128, 2], F32, tag="sums2")
            nc.vector.reduce_sum(out=sums2[:, 0:1], in_=partials[p], axis=AX.X)
            nc.vector.reduce_sum(out=sums2[:, 1:2], in_=partials[p], axis=AX.X)
            srow2 = pssm.tile([2, 128], F32, tag="ps_small")
            nc.tensor.transpose(out=srow2, in_=sums2, identity=ident)
            # logits = (a1*s + a3 ; a2*s + a4)^T @ (s ; 1)  -- rank-2 in s
            nc.scalar.copy(out=rhs_qk[0:1, :], in_=srow2[0:1, :])
            nc.vector.tensor_scalar(out=lhsT2, in0=srow2,
                                    scalar1=a12c, scalar2=a34c,
                                    op0=MUL, op1=ADD)
            lg_ps = pssm.tile([128, 128], F32, tag="ps_small")
            nc.tensor.matmul(out=lg_ps, lhsT=lhsT2, rhs=rhs_qk,
                             start=True, stop=True)
            # exp of each diagonal block; accum_out gives the softmax denom
            sumexp = pairbuf.tile([128, 1], F32, tag="sumexp")
            for h in range(2):
                r = slice(h * 64, h * 64 + 64)
                nc.scalar.activation(
                    out=attn[r, r], in_=lg_ps[r, r], func=AF.Exp,
                    accum_out=sumexp[r, :],
                )
            recip = pairbuf.tile([128, 1], F32, tag="recip")
            nc.vector.reciprocal(out=recip, in_=sumexp)
            nc.vector.tensor_scalar(out=attn, in0=attn, scalar1=recip,
                                    scalar2=alpha_bc, op0=MUL, op1=MUL)
            # lhsT = (I + alpha*attn)^T: transpose into PSUM, then
            # accumulate identity (ident^T @ ident = I) into the same bank
            at_ps = pssm.tile([128, 128], F32, tag="ps_small")
            nc.tensor.transpose(out=at_ps, in_=attn, identity=ident)
            lt = pairbuf.tile([128, 128], F32, tag="lhsTf")
            nc.vector.tensor_add(out=lt, in0=at_ps, in1=ident)
            ltr = pairbuf.tile([128, 128], F32R, tag="lhsTr")
            nc.scalar.copy(out=ltr, in_=lt)
            lhsT[p] = ltr

        def emit_compute(p, src, srcf, ncols, stores, drow, dcol_off,
                         after_group=None):
            # stores: list of (end_group, c0, c1); store k emitted right
            # after its covering copy-group lands
            ngrp = ncols // GCOLS
            by_group = {g: (c0, c1) for (g, c0, c1) in stores}
            for g in range(ngrp):
                if after_group and g in after_group:
                    after_group[g]()
                mm = psmm.tile([128, GROUP, 512], F32, tag="mm")
                base = g * GCOLS
                for j in range(GROUP):
                    nc.tensor.matmul(
                        out=mm[:, j, 0:CHUNK],
                        lhsT=lhsT[p],
                        rhs=src[:, base + j * CHUNK: base + (j + 1) * CHUNK],
                        start=True, stop=True,
                    )
                dst = src[:, base: base + GCOLS].rearrange(
                    "p (a c) -> p a c", a=GROUP)
                nc.scalar.copy(out=dst, in_=mm[:, :, 0:CHUNK])
                if g in by_group:
                    c0, c1 = by_group[g]
                    nc.scalar.dma_start(
                        out=out[drow:drow + 128, dcol_off + c0:dcol_off + c1],
                        in_=srcf[:, c0:c1])

        # ---- emission: defines per-engine queue order ----
        # pair0 loads (SP ring), consts after the first so x data leads
        c0, c1 = IN0[0]
        nc.sync.dma_start(out=A[:, c0:c1], in_=x[0:128, c0:c1])
        nc.sync.dma_start(out=cblk, in_=consts_d[:, :])
        nc.vector.reduce_sum(out=partials[0][:, 0:1], in_=Af[:, c0:c1],
                             axis=AX.X)
        scratch = consts.tile([128, 1], F32)
        # pre-load the ACT exp table off the critical path
        nc.scalar.activation(out=scratch, in_=alpha_bc, func=AF.Exp)
        for i, (c0, c1) in enumerate(IN0[1:], start=1):
            nc.sync.dma_start(out=A[:, c0:c1], in_=x[0:128, c0:c1])
            nc.vector.reduce_sum(out=partials[0][:, i:i + 1],
                                 in_=Af[:, c0:c1], axis=AX.X)
        emit_smalls(0)
        # pair1 B-region loads: region free immediately
        emit_loads(IN1B, Bt, Bf, 128, 0, partials[1], 0)

        # pair1 A-region loads ride the ACT ring, interleaved between
        # pair0's store triggers: same-ring FIFO guarantees their writes
        # to A[:, 0:12000] happen after pair0's stores read those columns
        # (cross-ring DMA WAR is NOT tracked by Tile), and the spread
        # keeps store backlog in the ring while each load's conservative
        # completion-wait resolves.
        def emit_in1a(i):
            def cb():
                emit_loads(IN1A[i:i + 1], A, Af, 128, TB, partials[1],
                           len(IN1B) + i, eng=nc.scalar)
            return cb

        # pair0 compute + stores (ACT ring)
        emit_compute(0, A, Af, T,
                     [(o1 // GCOLS - 1, o0, o1) for (o0, o1) in OUT0],
                     0, 0,
                     after_group={13: emit_in1a(0), 19: emit_in1a(1),
                                  21: emit_in1a(2)})
        emit_smalls(1)
        # pair1 compute: B region, then the A-region tail
        emit_compute(1, Bt, Bf, TB,
                     [(o1 // GCOLS - 1, o0, o1) for (o0, o1) in OUT1B],
                     128, 0)
        emit_compute(1, A, Af, T - TB,
                     [(o1 // GCOLS - 1, o0, o1) for (o0, o1) in OUT1A],
                     128, TB)

    # Bacc legalization: splits multi-wait sync into EventSemaphore
    # instructions (HW allows one wait per instruction) etc.
    nc.compile()
    return nc


def _host_inputs(x, Wq, bq, Wk, bk, Wv, bv, alpha):
    """Build per-core input maps.  Rank-2 logit coefficients, with the
    1/sqrt(D)=1/8 scale and the 1/T mean folded in:
    logits[c,e] = a1*S_c*S_e + a2*S_c + a3*S_e + a4, S = row sums of x.
    """
    x = np.ascontiguousarray(np.asarray(x, dtype=np.float32))
    wq = np.asarray(Wq)[:, 0].astype(np.float64)
    wk = np.asarray(Wk)[:, 0].astype(np.float64)
    bqv = np.asarray(bq).astype(np.float64)
    bkv = np.asarray(bk).astype(np.float64)
    cb = np.zeros((128, CONST_COLS), dtype=np.float32)
    cb[:, 0:128] = np.eye(128, dtype=np.float32)
    cb[:, 128] = np.float32(alpha)
    cb[0, 129] = wq @ wk / (8.0 * T * T)
    cb[1, 129] = wq @ bkv / (8.0 * T)
    cb[0, 130] = bqv @ wk / (8.0 * T)
    cb[1, 130] = bqv @ bkv / 8.0
    cb[1, 144:272] = 1.0
    in_maps = []
    for c in range(N_CORES):
        shard = x[c * BPC:(c + 1) * BPC].reshape(ROWS, T)
        in_maps.append({
            "x": np.ascontiguousarray(shard),
            "consts": cb,
        })
    return in_maps


def run(inputs: dict, trace: bool = False, tmpdir: str | None = None):
    nc = build_bass()
    in_maps = _host_inputs(**inputs)
    res = run_bass_kernel_spmd(
        nc, in_maps, core_ids=list(range(N_CORES)), trace=trace, tmpdir=tmpdir,
    )
    outs = [m["out"].reshape(BPC, C, T) for m in res.results]
    full = np.concatenate(outs, axis=0)
    return full, res


def kernel(**inputs) -> np.ndarray:
    full, _ = run(inputs, trace=bool(os.environ.get("C2C_TRACE")))
    return full


if __name__ == "__main__":
    # quick single-core numerical check in CoreSim
    from concourse import bass_interp

    rng = np.random.default_rng(0)
    x = rng.standard_normal((BPC, C, T), dtype=np.float32)
    Wq = rng.standard_normal((D, 1)).astype(np.float32)
    bq = rng.standard_normal((D,)).astype(np.float32)
    Wk = rng.standard_normal((D, 1)).astype(np.float32)
    bk = rng.standard_normal((D,)).astype(np.float32)
    alpha = np.float32(0.5)

    nc = build_bass()
    sim = bass_interp.CoreSim(nc)
    im = _host_inputs(x=np.tile(x, (N_CORES, 1, 1)), Wq=Wq, bq=bq, Wk=Wk, bk=bk,
                      Wv=None, bv=None, alpha=alpha)[0]
    for k, v in im.items():
        sim.tensor(k)[:] = v
    sim.simulate()
    got = np.asarray(sim.tensor("out")).reshape(BPC, C, T)

    desc = x.mean(axis=2, keepdims=True)
    q = desc * Wq[:, 0] + bq
    k = desc * Wk[:, 0] + bk
    logits = np.einsum('bcd,bed->bce', q, k) / np.sqrt(D)
    m = logits.max(axis=-1, keepdims=True)
    e = np.exp(logits - m)
    attn = e / e.sum(axis=-1, keepdims=True)
    mixed = np.einsum('bce,bet->bct', attn, x)
    want = x + alpha * mixed
    err = np.abs(got - want)
    rel = np.linalg.norm(got - want) / np.linalg.norm(want)
    print("max abs err:", err.max(), "rel:", rel)
